# revision 14
# baseline (speedup 1.0000x reference)
"""Bass/Trainium2 kernel for ExtractPatchesPosition (bilinear patch extraction).

Problem: padded_obj (B=2048, 128, 128, 1) f32, positions (B, 1, 2, C=4) ->
patches (B, 64, 64, 4): per (sample, channel), sample a translated 64x64 grid
out(r,c) = img(r + 32 + oy, c + 32 + ox) with bilinear interpolation.
|offset| <= 20 and margin 32, so samples never leave the image: the patch is
the (65 x 65) window at integer origin (y0, x0) = floor(32 + (oy, ox)) blended
with the separable 2-tap weights (fy, fx).

Sharding: pure data parallel, batch over 8 cores (256 samples/core).

HBM traffic is the roofline (~358 GB/s per NeuronCore), and SWDGE indirect
gathers only support ONE element offset per partition with a uniform
contiguous run (multi-entry offset APs are silently truncated to the first
entry by the compiler — verified on HW), so the per-channel window gathers
cannot crop columns and must read ~2x the image per channel pass. Both
directions are therefore halved by dtype instead:
  - the image is cast to bf16 on the HOST and gathered as bf16 (17.0 MB/core
    instead of 34.1 MB),
  - the output is computed/stored as bf16 (8.4 MB/core instead of 16.8) and
    upcast to f32 on the host. Bilinear blending of bf16-rounded inputs keeps
    rel err ~4e-3 vs the 2e-2 gate.
Total ~25.4 MB/core/iter -> ~71 us DMA floor.

Device layout (channel-planar; the key to DMA efficiency):
  - partitions = 128 SAMPLES of a group (2 groups/core); per (group, channel)
    ONE full-window gather (65 rows x 128 cols, one contiguous 16.6 KB bf16
    descriptor per partition — both data-dependent shifts absorbed into the
    element-granularity start offset), then 2 row-halves x 4 channels of
    blend passes interleave into two output tiles whose free dim is exactly
    the HBM layout (rows, cols, ch). No transpose, no PSUM.

Per (group g, channel ch):
  1. SWDGE indirect gather: per partition 65*128 bf16 from flat offset
     ((g*128+p)*128 + y0)*128 + x0.
  2. per half h:
     ACT:  u = fy * W[r+1, x]              (32x65, bf16->f32)
     DVE:  t = (1-fy)*W[r, x] + u          (32x65)   vertical 2-tap
     ACT:  v = fx * t[r, x+1]              (32x64)
     DVE:  o[r, x, ch] = (1-fx)*t[r,x] + v (32x64, ->bf16)
  3. after ch=3: two HWDGE stores (one per half) of
     out[g*128:(g+1)*128, 32h:32h+32, :, :]   (2MB bf16, 16KB descriptors)

Window metadata (int origins, fractional weights) is computed on host from
`positions` (O(B*C) work) and passed as small input tensors; all O(B*N*N*C)
data movement and math runs on device.
"""

import numpy as np
import ml_dtypes

import concourse.bacc as bacc
import concourse.tile as tile
from concourse import mybir
from concourse.bass import IndirectOffsetOnAxis

B, M, N, C = 2048, 128, 64, 4
NCORES = 8
BC = B // NCORES          # 256 samples per core
GS = 128                  # samples per group (one per partition)
GROUPS = BC // GS         # 2 groups
HALVES = 2                # row halves per group (blend/store granularity)
HR = N // HALVES          # 32 output rows per half
WR = N + 1                # 65 gathered window rows per (group, channel)
RUNW = WR * M             # 8320 gathered elements per gather unit
NG = GROUPS * C           # 8 gather units
NU = GROUPS * HALVES * C  # 16 blend units (meta granularity)
F32 = mybir.dt.float32
BF16 = mybir.dt.bfloat16
NP_BF16 = ml_dtypes.bfloat16
Copy = mybir.ActivationFunctionType.Copy
MULT = mybir.AluOpType.mult
ADD = mybir.AluOpType.add

_NC_CACHE = {}


def _build_nc(reps=1, loop_iters=1, do_gather=True, do_compute=True,
              do_store=True):
    """Build the Bass module. reps>1 replicates the whole pipeline inside the
    NEFF (same tiles, same output) and loop_iters>1 wraps that body in a
    hardware loop — both used only for steady-state timing (total iterations
    per NEFF execution = reps * loop_iters). The do_* flags disable pipeline
    stages for isolation micro-benchmarks (disabled stages read from
    once-initialized shared tiles so every access stays legal)."""
    nc = bacc.Bacc("TRN2")
    img = nc.declare_dram_parameter("img", [BC * M * M, 1], BF16, isOutput=False)
    idx = nc.declare_dram_parameter("idx", [128, NG], mybir.dt.int32, isOutput=False)
    meta = nc.declare_dram_parameter("meta", [128, 4 * NU], F32, isOutput=False)
    out = nc.declare_dram_parameter("out", [BC, N, N, C], BF16, isOutput=True)

    with tile.TileContext(nc) as tc:
        with (
            tc.tile_pool(name="singles", bufs=1) as singles,
            tc.tile_pool(name="gpool", bufs=3) as gpool,
            tc.tile_pool(name="upool", bufs=4) as upool,
            tc.tile_pool(name="tpool", bufs=4) as tpool,
            tc.tile_pool(name="vpool", bufs=4) as vpool,
            tc.tile_pool(name="opool", bufs=4) as opool,
        ):
            idx_sb = singles.tile([128, NG], mybir.dt.int32)
            meta_sb = singles.tile([128, 4 * NU], F32)
            nc.sync.dma_start(idx_sb[:], idx[:])
            nc.sync.dma_start(meta_sb[:], meta[:])
            if not do_gather:
                G_shared = singles.tile([128, RUNW], BF16)
                nc.sync.dma_start(
                    G_shared[:], img[: 128 * RUNW, :].rearrange("(p x) 1 -> p x", p=128)
                )
            if not do_compute:
                o_shared = [
                    singles.tile([128, HR * N * C], BF16, name=f"osh{h}")
                    for h in range(HALVES)
                ]
                for h in range(HALVES):
                    nc.sync.dma_start(
                        o_shared[h][:],
                        img[: 128 * HR * N * C, :].rearrange(
                            "(p x) 1 -> p x", p=128
                        ),
                    )

            def _body():
                for rep in range(reps):
                    for g in range(GROUPS):
                        if do_compute:
                            o_tiles = [
                                opool.tile(
                                    [128, HR * N * C], BF16, tag="o", name=f"o{h}"
                                )
                                for h in range(HALVES)
                            ]
                        else:
                            o_tiles = o_shared
                        for ch in range(C):
                            gi = g * C + ch
                            if do_gather:
                                G = gpool.tile([128, RUNW], BF16, tag="G")
                                nc.gpsimd.indirect_dma_start(
                                    out=G[:],
                                    out_offset=None,
                                    in_=img[:],
                                    in_offset=IndirectOffsetOnAxis(
                                        ap=idx_sb[:, gi : gi + 1], axis=0
                                    ),
                                )
                            else:
                                G = G_shared
                            Gv = G[:].rearrange("p (r x) -> p r x", x=M)

                            for h in range(HALVES if do_compute else 0):
                                u_idx = (g * HALVES + h) * C + ch
                                fy1 = meta_sb[:, 4 * u_idx + 0 : 4 * u_idx + 1]
                                fy = meta_sb[:, 4 * u_idx + 1 : 4 * u_idx + 2]
                                fx1 = meta_sb[:, 4 * u_idx + 2 : 4 * u_idx + 3]
                                fx = meta_sb[:, 4 * u_idx + 3 : 4 * u_idx + 4]
                                r0 = h * HR

                                # vertical 2-tap: t = (1-fy)*W[r] + fy*W[r+1]
                                # u/t are bf16 and 66 wide (even extent,
                                # 4B-aligned rows) so the DVE stt runs in
                                # 2x packed mode.
                                TP = N + 2
                                u = upool.tile([128, HR * TP], BF16, tag="u")
                                uv = u[:].rearrange("p (r x) -> p r x", x=TP)
                                nc.scalar.activation(
                                    uv,
                                    Gv[:, r0 + 1 : r0 + HR + 1, 0:TP],
                                    Copy,
                                    scale=fy,
                                )
                                t = tpool.tile([128, HR * TP], BF16, tag="t")
                                tv = t[:].rearrange("p (r x) -> p r x", x=TP)
                                nc.vector.scalar_tensor_tensor(
                                    tv,
                                    Gv[:, r0 : r0 + HR, 0:TP],
                                    fy1,
                                    uv,
                                    MULT,
                                    ADD,
                                )

                                # horizontal 2-tap: o = (1-fx)*t[x] + fx*t[x+1]
                                v = vpool.tile([128, HR * N], BF16, tag="v")
                                vv = v[:].rearrange("p (r x) -> p r x", x=N)
                                nc.scalar.activation(
                                    vv, tv[:, :, 1 : N + 1], Copy, scale=fx
                                )
                                ov = o_tiles[h][:].rearrange(
                                    "p (r c ch) -> p r c ch", c=N, ch=C
                                )
                                nc.vector.scalar_tensor_tensor(
                                    ov[:, :, :, ch], tv[:, :, 0:N], fx1, vv,
                                    MULT, ADD,
                                )

                        for h in range(HALVES if do_store else 0):
                            dst = out[
                                g * GS : (g + 1) * GS, h * HR : (h + 1) * HR, :, :
                            ]
                            # Stores issue from the sync (SP) engine only:
                            # a store on nc.scalar would make the ACT queue
                            # stall on the o-tile semaphore, blocking blend
                            # compute behind it.
                            nc.sync.dma_start(
                                out=dst,
                                in_=o_tiles[h][:].rearrange(
                                    "p (r c ch) -> p r c ch", c=N, ch=C
                                ),
                            )

            if loop_iters > 1:
                with tc.For_i(0, loop_iters):
                    _body()
            else:
                _body()
    nc.finalize()
    return nc


def get_nc():
    if "nc" not in _NC_CACHE:
        _NC_CACHE["nc"] = _build_nc()
    return _NC_CACHE["nc"]


def make_core_inputs(padded_obj, positions):
    """Host-side prep: shard + window metadata. Returns list of in_maps."""
    padded_obj = np.asarray(padded_obj, dtype=np.float32)
    positions = np.asarray(positions, dtype=np.float32)
    ox = positions[:, 0, 0, :]  # [B, C] column offsets
    oy = positions[:, 0, 1, :]  # [B, C] row offsets
    c0 = np.float32((M - N) // 2)
    sx = (c0 + ox).astype(np.float32)
    sy = (c0 + oy).astype(np.float32)
    x0 = np.floor(sx).astype(np.int32)
    y0 = np.floor(sy).astype(np.int32)
    fx = (sx - x0.astype(np.float32)).astype(np.float32)
    fy = (sy - y0.astype(np.float32)).astype(np.float32)

    p = np.arange(GS)
    in_maps = []
    for core in range(NCORES):
        s = slice(core * BC, (core + 1) * BC)
        img_c = (
            np.ascontiguousarray(padded_obj[s, :, :, 0])
            .reshape(-1, 1)
            .astype(NP_BF16)
        )
        y0c, x0c = y0[s], x0[s]
        fyc, fxc = fy[s], fx[s]
        idx_c = np.empty((128, NG), np.int32)
        meta_c = np.empty((128, 4 * NU), np.float32)
        for g in range(GROUPS):
            b_loc = g * GS + p  # [128] sample index within core
            for ch in range(C):
                idx_c[:, g * C + ch] = (
                    b_loc * M + y0c[b_loc, ch]
                ) * M + x0c[b_loc, ch]
                for h in range(HALVES):
                    u_idx = (g * HALVES + h) * C + ch
                    meta_c[:, 4 * u_idx + 0] = np.float32(1.0) - fyc[b_loc, ch]
                    meta_c[:, 4 * u_idx + 1] = fyc[b_loc, ch]
                    meta_c[:, 4 * u_idx + 2] = np.float32(1.0) - fxc[b_loc, ch]
                    meta_c[:, 4 * u_idx + 3] = fxc[b_loc, ch]
        in_maps.append({"img": img_c, "idx": idx_c, "meta": meta_c})
    return in_maps


def _make_runner(nc):
    """Build a persistent jitted SPMD executor for `nc` (compiles once).

    Mirrors concourse.bass2jax.run_bass_via_pjrt but caches the jitted
    function and keeps the zero output buffers resident on device, so
    repeated kernel() calls ship only the actual inputs.
    """
    import jax
    from jax.sharding import Mesh, PartitionSpec, NamedSharding
    from jax.experimental.shard_map import shard_map
    from concourse import bass2jax, mybir as mb

    bass2jax.install_neuronx_cc_hook()
    assert not nc.dbg_callbacks, "dbg callbacks unsupported under axon"

    extra_in_maps = {}
    if nc.dbg_addr is not None:
        extra_in_maps[nc.dbg_addr.name] = np.zeros((1, 2), np.uint32)
    partition_name = nc.partition_id_tensor.name if nc.partition_id_tensor else None

    in_names, out_names, out_avals = [], [], []
    for alloc in nc.m.functions[0].allocations:
        if not isinstance(alloc, mb.MemoryLocationSet):
            continue
        name = alloc.memorylocations[0].name
        if alloc.kind == "ExternalInput":
            if name != partition_name:
                in_names.append(name)
        elif alloc.kind == "ExternalOutput":
            out_names.append(name)
            out_avals.append(
                jax.core.ShapedArray(tuple(alloc.tensor_shape), mb.dt.np(alloc.dtype))
            )
    n_params = len(in_names)
    n_outs = len(out_avals)
    all_names = in_names + out_names
    if partition_name is not None:
        all_names = all_names + [partition_name]

    def _body(*args):
        operands = list(args)
        if partition_name is not None:
            operands.append(bass2jax.partition_id_tensor())
        outs = bass2jax._bass_exec_p.bind(
            *operands,
            out_avals=tuple(out_avals),
            in_names=tuple(all_names),
            out_names=tuple(out_names),
            lowering_input_output_aliases=(),
            sim_require_finite=True,
            sim_require_nnan=True,
            nc=nc,
        )
        return tuple(outs)

    devices = jax.devices()[:NCORES]
    mesh = Mesh(np.asarray(devices), ("core",))
    in_specs = (PartitionSpec("core"),) * (n_params + n_outs)
    out_specs = (PartitionSpec("core"),) * n_outs
    sharded = jax.jit(
        shard_map(_body, mesh=mesh, in_specs=in_specs, out_specs=out_specs,
                  check_rep=False),
        keep_unused=True,
    )
    sh = NamedSharding(mesh, PartitionSpec("core"))
    zeros_cache = {}

    def _device_zeros():
        # The NEFF writes every output element; the zero operands are only
        # buffer placeholders. Keep them resident on device across calls.
        if "z" not in zeros_cache:
            zeros_cache["z"] = [
                jax.device_put(
                    np.zeros((NCORES * a.shape[0], *a.shape[1:]), a.dtype), sh
                )
                for a in out_avals
            ]
        return zeros_cache["z"]

    def run(in_maps, device_only=False, device_inputs=None):
        if device_inputs is None:
            if extra_in_maps:
                in_maps = [{**m, **extra_in_maps} for m in in_maps]
            device_inputs = [
                np.concatenate([np.asarray(m[name]) for m in in_maps], axis=0)
                for name in in_names
            ] + _device_zeros()
        out_arrs = sharded(*device_inputs)
        if device_only:
            jax.block_until_ready(out_arrs)
            return None
        return {name: np.asarray(out_arrs[i]) for i, name in enumerate(out_names)}

    def put_inputs(in_maps):
        """Place concat inputs + zero outputs on device once (for benching)."""
        if extra_in_maps:
            in_maps = [{**m, **extra_in_maps} for m in in_maps]
        arrs = [
            jax.device_put(
                np.concatenate([np.asarray(m[name]) for m in in_maps], axis=0), sh
            )
            for name in in_names
        ] + _device_zeros()
        jax.block_until_ready(arrs)
        return arrs

    run.put_inputs = put_inputs
    return run


def get_runner():
    if "run" not in _NC_CACHE:
        _NC_CACHE["run"] = _make_runner(get_nc())
    return _NC_CACHE["run"]


def kernel(padded_obj, positions, N=None):
    assert padded_obj.shape == (B, M, M, 1), padded_obj.shape
    in_maps = make_core_inputs(padded_obj, positions)
    out = get_runner()(in_maps)["out"]
    return np.ascontiguousarray(out.astype(np.float32))


# revision 18
# speedup vs baseline: 1.2108x; 1.2108x over previous
"""Bass/Trainium2 kernel for ExtractPatchesPosition (bilinear patch extraction).

Problem: padded_obj (B=2048, 128, 128, 1) f32, positions (B, 1, 2, C=4) ->
patches (B, 64, 64, 4): per (sample, channel), sample a translated 64x64 grid
out(r,c) = img(r + 32 + oy, c + 32 + ox) with bilinear interpolation.
|offset| <= 20 and margin 32, so samples never leave the image: the patch is
the (65 x 65) window at integer origin (y0, x0) = floor(32 + (oy, ox)) blended
with the separable 2-tap weights (fy, fx).

Sharding: pure data parallel, batch over 8 cores (256 samples/core).

Design notes (what measurement established):
  - HBM (~358 GB/s/NC) is the roofline. SWDGE indirect gathers support only
    ONE element offset per partition with a uniform contiguous run (verified
    on HW: multi-entry offset APs silently use entry 0), so per-channel
    window gathers can't crop columns; read traffic is fixed at ~2x the
    image per channel sweep. Both DMA directions are instead halved by
    dtype: the image is cast to bf16 on the HOST and gathered as bf16
    (17.0 MB/core, measured full-rate ~340 GB/s), and the output is stored
    bf16 (8.4 MB/core) then upcast on the host. End-to-end bf16 rounding
    keeps rel err ~7e-3 vs the 2e-2 gate.
  - DVE is NOT viable for the blend at this size: scalar_tensor_tensor has
    no 2x packed mode (measured 1 el/cycle @0.96 GHz) and the
    channel-interleaved output write is ~2x slower still -> 111 us/iter on
    DVE alone. The blend instead runs on the TENSOR engine as 4 accumulating
    diagonal matmuls per (group, channel): a diag(w) stationary matrix is a
    per-partition scalar multiply, PSUM accumulates the 4 bilinear taps, and
    ACT (stride-agnostic, 1 el/cycle @1.2 GHz) evacuates PSUM into the
    channel-interleaved bf16 output tile.

Per (group g, channel ch):
  1. SWDGE indirect gather: per partition 65*128 bf16 from flat offset
     ((g*128+p)*128 + y0)*128 + x0 (one 16.6 KB descriptor per partition,
     both data-dependent shifts absorbed into the element-granularity start).
  2. per half h, per PSUM chunk (16 rows): 4 matmuls
     psum[r,c] += diag(w_tap) @ G[r0+dr+r, dc+c], taps (dr,dc) in {0,1}^2,
     w_tap = host-computed {(1-fy)(1-fx), (1-fy)fx, fy(1-fx), fy fx}.
  3. ACT evacuates psum (32x64) -> o[g][:, r, c, ch] (bf16, interleaved).
  4. after ch=3: two HWDGE stores (one per half) of
     out[g*128:(g+1)*128, 32h:32h+32, :, :]   (2MB bf16, 16KB descriptors)

Window metadata (int origins, per-tap diagonal weights) is computed on host
from `positions` (O(B*C) work) and passed as small input tensors; all
O(B*N*N*C) data movement and math runs on device.
"""

import numpy as np
import ml_dtypes

import concourse.bacc as bacc
import concourse.tile as tile
from concourse import mybir
from concourse.bass import IndirectOffsetOnAxis

B, M, N, C = 2048, 128, 64, 4
NCORES = 8
BC = B // NCORES          # 256 samples per core
GS = 128                  # samples per group (one per partition)
GROUPS = BC // GS         # 2 groups
HALVES = 2                # row halves per group (PSUM/store granularity)
HR = N // HALVES          # 32 output rows per half
WR = N + 1                # 65 gathered window rows per (group, channel)
RUNW = WR * M             # 8320 gathered elements per gather unit
NG = GROUPS * C           # 8 gather units
TAPS = 4                  # bilinear taps (dr, dc) in {0,1}^2
MMF = 512                 # matmul free-element limit (s3d3_mm_num_elements)
CHUNK_R = MMF // N        # 8 window rows per matmul chunk
F32 = mybir.dt.float32
BF16 = mybir.dt.bfloat16
NP_BF16 = ml_dtypes.bfloat16
Copy = mybir.ActivationFunctionType.Copy

_NC_CACHE = {}


def _build_nc(reps=1, loop_iters=1, do_gather=True, do_compute=True,
              do_store=True):
    """Build the Bass module. reps>1 replicates the whole pipeline inside the
    NEFF (same tiles, same output) and loop_iters>1 wraps that body in a
    hardware loop — both used only for steady-state timing (total iterations
    per NEFF execution = reps * loop_iters). The do_* flags disable pipeline
    stages for isolation micro-benchmarks (disabled stages read from
    once-initialized shared tiles so every access stays legal)."""
    nc = bacc.Bacc("TRN2")
    img = nc.declare_dram_parameter("img", [BC * M * M, 1], BF16, isOutput=False)
    idx = nc.declare_dram_parameter("idx", [128, NG], mybir.dt.int32, isOutput=False)
    wdiag = nc.declare_dram_parameter(
        "wdiag", [128, NG * TAPS * 128], BF16, isOutput=False
    )
    out = nc.declare_dram_parameter("out", [BC, N, N, C], BF16, isOutput=True)

    with tile.TileContext(nc) as tc:
        with (
            tc.tile_pool(name="singles", bufs=1) as singles,
            tc.tile_pool(name="gpool", bufs=3) as gpool,
            tc.tile_pool(name="opool", bufs=2) as opool,
            tc.tile_pool(name="ppool", bufs=2, space="PSUM") as ppool,
        ):
            idx_sb = singles.tile([128, NG], mybir.dt.int32)
            w_sb = singles.tile([128, NG * TAPS * 128], BF16)
            nc.sync.dma_start(idx_sb[:], idx[:])
            nc.sync.dma_start(w_sb[:], wdiag[:])
            if not do_gather:
                G_shared = singles.tile([128, RUNW], BF16)
                nc.sync.dma_start(
                    G_shared[:],
                    img[: 128 * RUNW, :].rearrange("(p x) 1 -> p x", p=128),
                )
            if not do_compute:
                o_shared = [
                    singles.tile([128, HR * N * C], BF16, name=f"osh{h}")
                    for h in range(HALVES)
                ]
                for h in range(HALVES):
                    nc.sync.dma_start(
                        o_shared[h][:],
                        img[: 128 * HR * N * C, :].rearrange(
                            "(p x) 1 -> p x", p=128
                        ),
                    )

            def _body():
                for rep in range(reps):
                    for g in range(GROUPS):
                        if do_compute:
                            o_tiles = [
                                opool.tile(
                                    [128, HR * N * C], BF16, tag="o", name=f"o{h}"
                                )
                                for h in range(HALVES)
                            ]
                        else:
                            o_tiles = o_shared
                        for ch in range(C):
                            gi = g * C + ch
                            if do_gather:
                                G = gpool.tile([128, RUNW], BF16, tag="G")
                                nc.gpsimd.indirect_dma_start(
                                    out=G[:],
                                    out_offset=None,
                                    in_=img[:],
                                    in_offset=IndirectOffsetOnAxis(
                                        ap=idx_sb[:, gi : gi + 1], axis=0
                                    ),
                                )
                            else:
                                G = G_shared
                            Gv = G[:].rearrange("p (r x) -> p r x", x=M)

                            for h in range(HALVES if do_compute else 0):
                                r0 = h * HR
                                ps = ppool.tile([128, HR * N], F32, tag="ps")
                                psv = ps[:].rearrange("p (r x) -> p r x", x=N)
                                for ck in range(HR // CHUNK_R):
                                    cr = r0 + ck * CHUNK_R
                                    for tap in range(TAPS):
                                        dr, dc = tap >> 1, tap & 1
                                        w_ap = w_sb[
                                            :,
                                            (gi * TAPS + tap) * 128 :
                                            (gi * TAPS + tap + 1) * 128,
                                        ]
                                        nc.tensor.matmul(
                                            out=psv[
                                                :,
                                                ck * CHUNK_R : (ck + 1) * CHUNK_R,
                                                :,
                                            ],
                                            lhsT=w_ap,
                                            rhs=Gv[
                                                :,
                                                cr + dr : cr + dr + CHUNK_R,
                                                dc : dc + N,
                                            ],
                                            start=(tap == 0),
                                            stop=(tap == TAPS - 1),
                                        )
                                ov = o_tiles[h][:].rearrange(
                                    "p (r c ch) -> p r c ch", c=N, ch=C
                                )
                                # Evacuate PSUM -> interleaved bf16 output.
                                # Alternate ACT/DVE so neither engine's
                                # stride-penalized writes become critical.
                                if ch % 2 == 0:
                                    nc.scalar.activation(
                                        ov[:, :, :, ch], psv, Copy, scale=1.0
                                    )
                                else:
                                    nc.vector.tensor_copy(
                                        out=ov[:, :, :, ch], in_=psv
                                    )

                        for h in range(HALVES if do_store else 0):
                            dst = out[
                                g * GS : (g + 1) * GS, h * HR : (h + 1) * HR, :, :
                            ]
                            # Stores issue from the sync (SP) engine only: a
                            # store on nc.scalar would make the ACT queue
                            # stall on the o-tile semaphore, blocking the
                            # PSUM evacuations behind it.
                            nc.sync.dma_start(
                                out=dst,
                                in_=o_tiles[h][:].rearrange(
                                    "p (r c ch) -> p r c ch", c=N, ch=C
                                ),
                            )

            if loop_iters > 1:
                with tc.For_i(0, loop_iters):
                    _body()
            else:
                _body()
    nc.finalize()
    return nc


def get_nc():
    if "nc" not in _NC_CACHE:
        _NC_CACHE["nc"] = _build_nc()
    return _NC_CACHE["nc"]


def make_core_inputs(padded_obj, positions):
    """Host-side prep: shard + window metadata. Returns list of in_maps."""
    padded_obj = np.asarray(padded_obj, dtype=np.float32)
    positions = np.asarray(positions, dtype=np.float32)
    ox = positions[:, 0, 0, :]  # [B, C] column offsets
    oy = positions[:, 0, 1, :]  # [B, C] row offsets
    c0 = np.float32((M - N) // 2)
    sx = (c0 + ox).astype(np.float32)
    sy = (c0 + oy).astype(np.float32)
    x0 = np.floor(sx).astype(np.int32)
    y0 = np.floor(sy).astype(np.int32)
    fx = (sx - x0.astype(np.float32)).astype(np.float32)
    fy = (sy - y0.astype(np.float32)).astype(np.float32)

    p = np.arange(GS)
    in_maps = []
    for core in range(NCORES):
        s = slice(core * BC, (core + 1) * BC)
        img_c = (
            np.ascontiguousarray(padded_obj[s, :, :, 0])
            .reshape(-1, 1)
            .astype(NP_BF16)
        )
        y0c, x0c = y0[s], x0[s]
        fyc, fxc = fy[s], fx[s]
        idx_c = np.empty((128, NG), np.int32)
        w_c = np.zeros((128, NG * TAPS * 128), np.float32)
        for g in range(GROUPS):
            b_loc = g * GS + p  # [128] sample index within core
            for ch in range(C):
                gi = g * C + ch
                idx_c[:, gi] = (b_loc * M + y0c[b_loc, ch]) * M + x0c[b_loc, ch]
                fyv, fxv = fyc[b_loc, ch], fxc[b_loc, ch]
                taps = [
                    (1.0 - fyv) * (1.0 - fxv),
                    (1.0 - fyv) * fxv,
                    fyv * (1.0 - fxv),
                    fyv * fxv,
                ]
                for tap in range(TAPS):
                    w_c[p, (gi * TAPS + tap) * 128 + p] = taps[tap]
        in_maps.append(
            {"img": img_c, "idx": idx_c, "wdiag": w_c.astype(NP_BF16)}
        )
    return in_maps


def _make_runner(nc):
    """Build a persistent jitted SPMD executor for `nc` (compiles once).

    Mirrors concourse.bass2jax.run_bass_via_pjrt but caches the jitted
    function and keeps the zero output buffers resident on device, so
    repeated kernel() calls ship only the actual inputs.
    """
    import jax
    from jax.sharding import Mesh, PartitionSpec, NamedSharding
    from jax.experimental.shard_map import shard_map
    from concourse import bass2jax, mybir as mb

    bass2jax.install_neuronx_cc_hook()
    assert not nc.dbg_callbacks, "dbg callbacks unsupported under axon"

    extra_in_maps = {}
    if nc.dbg_addr is not None:
        extra_in_maps[nc.dbg_addr.name] = np.zeros((1, 2), np.uint32)
    partition_name = nc.partition_id_tensor.name if nc.partition_id_tensor else None

    in_names, out_names, out_avals = [], [], []
    for alloc in nc.m.functions[0].allocations:
        if not isinstance(alloc, mb.MemoryLocationSet):
            continue
        name = alloc.memorylocations[0].name
        if alloc.kind == "ExternalInput":
            if name != partition_name:
                in_names.append(name)
        elif alloc.kind == "ExternalOutput":
            out_names.append(name)
            out_avals.append(
                jax.core.ShapedArray(tuple(alloc.tensor_shape), mb.dt.np(alloc.dtype))
            )
    n_params = len(in_names)
    n_outs = len(out_avals)
    all_names = in_names + out_names
    if partition_name is not None:
        all_names = all_names + [partition_name]

    def _body(*args):
        operands = list(args)
        if partition_name is not None:
            operands.append(bass2jax.partition_id_tensor())
        outs = bass2jax._bass_exec_p.bind(
            *operands,
            out_avals=tuple(out_avals),
            in_names=tuple(all_names),
            out_names=tuple(out_names),
            lowering_input_output_aliases=(),
            sim_require_finite=True,
            sim_require_nnan=True,
            nc=nc,
        )
        return tuple(outs)

    devices = jax.devices()[:NCORES]
    mesh = Mesh(np.asarray(devices), ("core",))
    in_specs = (PartitionSpec("core"),) * (n_params + n_outs)
    out_specs = (PartitionSpec("core"),) * n_outs
    sharded = jax.jit(
        shard_map(_body, mesh=mesh, in_specs=in_specs, out_specs=out_specs,
                  check_rep=False),
        keep_unused=True,
    )
    sh = NamedSharding(mesh, PartitionSpec("core"))
    zeros_cache = {}

    def _device_zeros():
        # The NEFF writes every output element; the zero operands are only
        # buffer placeholders. Keep them resident on device across calls.
        if "z" not in zeros_cache:
            zeros_cache["z"] = [
                jax.device_put(
                    np.zeros((NCORES * a.shape[0], *a.shape[1:]), a.dtype), sh
                )
                for a in out_avals
            ]
        return zeros_cache["z"]

    def run(in_maps, device_only=False, device_inputs=None):
        if device_inputs is None:
            if extra_in_maps:
                in_maps = [{**m, **extra_in_maps} for m in in_maps]
            device_inputs = [
                np.concatenate([np.asarray(m[name]) for m in in_maps], axis=0)
                for name in in_names
            ] + _device_zeros()
        out_arrs = sharded(*device_inputs)
        if device_only:
            jax.block_until_ready(out_arrs)
            return None
        return {name: np.asarray(out_arrs[i]) for i, name in enumerate(out_names)}

    def put_inputs(in_maps):
        """Place concat inputs + zero outputs on device once (for benching)."""
        if extra_in_maps:
            in_maps = [{**m, **extra_in_maps} for m in in_maps]
        arrs = [
            jax.device_put(
                np.concatenate([np.asarray(m[name]) for m in in_maps], axis=0), sh
            )
            for name in in_names
        ] + _device_zeros()
        jax.block_until_ready(arrs)
        return arrs

    run.put_inputs = put_inputs
    return run


def get_runner():
    if "run" not in _NC_CACHE:
        _NC_CACHE["run"] = _make_runner(get_nc())
    return _NC_CACHE["run"]


def kernel(padded_obj, positions, N=None):
    assert padded_obj.shape == (B, M, M, 1), padded_obj.shape
    in_maps = make_core_inputs(padded_obj, positions)
    out = get_runner()(in_maps)["out"]
    return np.ascontiguousarray(out.astype(np.float32))


# revision 23
# speedup vs baseline: 1.2157x; 1.0040x over previous
"""Bass/Trainium2 kernel for ExtractPatchesPosition (bilinear patch extraction).

Problem: padded_obj (B=2048, 128, 128, 1) f32, positions (B, 1, 2, C=4) ->
patches (B, 64, 64, 4): per (sample, channel), sample a translated 64x64 grid
out(r,c) = img(r + 32 + oy, c + 32 + ox) with bilinear interpolation.
|offset| <= 20 and margin 32, so samples never leave the image: the patch is
the (65 x 65) window at integer origin (y0, x0) = floor(32 + (oy, ox)) blended
with the separable 2-tap weights (fy, fx).

Sharding: pure data parallel, batch over 8 cores (256 samples/core).

Design notes (what measurement established):
  - HBM (~358 GB/s/NC) is the roofline. SWDGE indirect gathers support only
    ONE element offset per partition with a uniform contiguous run (verified
    on HW: multi-entry offset APs silently use entry 0), so per-channel
    window gathers can't crop columns; read traffic is fixed at ~2x the
    image per channel sweep. Both DMA directions are instead halved by
    dtype: the image is cast to bf16 on the HOST and gathered as bf16
    (17.0 MB/core, measured full-rate ~340 GB/s), and the output is stored
    bf16 (8.4 MB/core) then upcast on the host. End-to-end bf16 rounding
    keeps rel err ~7e-3 vs the 2e-2 gate.
  - DVE is NOT viable for the blend at this size: scalar_tensor_tensor has
    no 2x packed mode (measured 1 el/cycle @0.96 GHz) and the
    channel-interleaved output write is ~2x slower still -> 111 us/iter on
    DVE alone. The blend instead runs on the TENSOR engine as 4 accumulating
    diagonal matmuls per (group, channel): a diag(w) stationary matrix is a
    per-partition scalar multiply, PSUM accumulates the 4 bilinear taps, and
    ACT (stride-agnostic, 1 el/cycle @1.2 GHz) evacuates PSUM into the
    channel-interleaved bf16 output tile.

Per (group g, channel ch):
  1. SWDGE indirect gather: per partition 65*128 bf16 from flat offset
     ((g*128+p)*128 + y0)*128 + x0 (one 16.6 KB descriptor per partition,
     both data-dependent shifts absorbed into the element-granularity start).
  2. per half h, per PSUM chunk (16 rows): 4 matmuls
     psum[r,c] += diag(w_tap) @ G[r0+dr+r, dc+c], taps (dr,dc) in {0,1}^2,
     w_tap = host-computed {(1-fy)(1-fx), (1-fy)fx, fy(1-fx), fy fx}.
  3. ACT evacuates psum (32x64) -> o[g][:, r, c, ch] (bf16, interleaved).
  4. after ch=3: two HWDGE stores (one per half) of
     out[g*128:(g+1)*128, 32h:32h+32, :, :]   (2MB bf16, 16KB descriptors)

Window metadata (int origins, per-tap diagonal weights) is computed on host
from `positions` (O(B*C) work) and passed as small input tensors; all
O(B*N*N*C) data movement and math runs on device.
"""

import numpy as np
import ml_dtypes

import concourse.bacc as bacc
import concourse.tile as tile
from concourse import mybir
from concourse.bass import IndirectOffsetOnAxis

B, M, N, C = 2048, 128, 64, 4
NCORES = 8
BC = B // NCORES          # 256 samples per core
GS = 128                  # samples per group (one per partition)
GROUPS = BC // GS         # 2 groups
HALVES = 2                # row halves per group (PSUM/store granularity)
HR = N // HALVES          # 32 output rows per half
WR = N + 1                # 65 gathered window rows per (group, channel)
# Window origins are bounded: y0, x0 = floor(32 + offset) with |offset|<=20,
# so every 65x65 window lies inside rows/cols [12, 117) of the 128x128 image.
# The host crops to that 105x105 region before shipping; gather runs shrink
# from 65*128 to 64*105+65 elements (-18% HBM reads).
CROP0 = 12                # first row/col of the cropped region
MC = 105                  # cropped image width/height
RUNW = WR * MC            # 6825 gathered elements per gather unit
PAD = 64                  # DRAM tail pad: last sample's max-offset window
                          # run reads up to 40 els past the cropped image
NG = GROUPS * C           # 8 gather units
TAPS = 4                  # bilinear taps (dr, dc) in {0,1}^2
MMF = 512                 # matmul free-element limit (s3d3_mm_num_elements)
CHUNK_R = MMF // N        # 8 window rows per matmul chunk
F32 = mybir.dt.float32
BF16 = mybir.dt.bfloat16
NP_BF16 = ml_dtypes.bfloat16
Copy = mybir.ActivationFunctionType.Copy

_NC_CACHE = {}


def _build_nc(reps=1, loop_iters=1, do_gather=True, do_compute=True,
              do_store=True):
    """Build the Bass module. reps>1 replicates the whole pipeline inside the
    NEFF (same tiles, same output) and loop_iters>1 wraps that body in a
    hardware loop — both used only for steady-state timing (total iterations
    per NEFF execution = reps * loop_iters). The do_* flags disable pipeline
    stages for isolation micro-benchmarks (disabled stages read from
    once-initialized shared tiles so every access stays legal)."""
    nc = bacc.Bacc("TRN2")
    img = nc.declare_dram_parameter(
        "img", [BC * MC * MC + PAD, 1], BF16, isOutput=False
    )
    idx = nc.declare_dram_parameter("idx", [128, NG], mybir.dt.int32, isOutput=False)
    wdiag = nc.declare_dram_parameter(
        "wdiag", [128, NG * TAPS * 128], BF16, isOutput=False
    )
    out = nc.declare_dram_parameter("out", [BC, N, N, C], BF16, isOutput=True)

    with tile.TileContext(nc) as tc:
        with (
            tc.tile_pool(name="singles", bufs=1) as singles,
            tc.tile_pool(name="gpool", bufs=3) as gpool,
            tc.tile_pool(name="opool", bufs=2) as opool,
            tc.tile_pool(name="ppool", bufs=2, space="PSUM") as ppool,
        ):
            idx_sb = singles.tile([128, NG], mybir.dt.int32)
            w_sb = singles.tile([128, NG * TAPS * 128], BF16)
            nc.sync.dma_start(idx_sb[:], idx[:])
            nc.sync.dma_start(w_sb[:], wdiag[:])
            if not do_gather:
                G_shared = singles.tile([128, RUNW], BF16)
                nc.sync.dma_start(
                    G_shared[:],
                    img[: 128 * RUNW, :].rearrange("(p x) 1 -> p x", p=128),
                )
            if not do_compute:
                o_shared = [
                    singles.tile([128, HR * N * C], BF16, name=f"osh{h}")
                    for h in range(HALVES)
                ]
                for h in range(HALVES):
                    nc.sync.dma_start(
                        o_shared[h][:],
                        img[: 128 * HR * N * C, :].rearrange(
                            "(p x) 1 -> p x", p=128
                        ),
                    )

            def _body():
                for rep in range(reps):
                    for g in range(GROUPS):
                        if do_compute:
                            o_tiles = [
                                opool.tile(
                                    [128, HR * N * C], BF16, tag="o", name=f"o{h}"
                                )
                                for h in range(HALVES)
                            ]
                        else:
                            o_tiles = o_shared
                        for ch in range(C):
                            gi = g * C + ch
                            if do_gather:
                                G = gpool.tile([128, RUNW], BF16, tag="G")
                                nc.gpsimd.indirect_dma_start(
                                    out=G[:],
                                    out_offset=None,
                                    in_=img[:],
                                    in_offset=IndirectOffsetOnAxis(
                                        ap=idx_sb[:, gi : gi + 1], axis=0
                                    ),
                                )
                            else:
                                G = G_shared
                            Gv = G[:].rearrange("p (r x) -> p r x", x=MC)

                            for h in range(HALVES if do_compute else 0):
                                r0 = h * HR
                                ps = ppool.tile([128, HR * N], F32, tag="ps")
                                psv = ps[:].rearrange("p (r x) -> p r x", x=N)
                                for ck in range(HR // CHUNK_R):
                                    cr = r0 + ck * CHUNK_R
                                    for tap in range(TAPS):
                                        dr, dc = tap >> 1, tap & 1
                                        w_ap = w_sb[
                                            :,
                                            (gi * TAPS + tap) * 128 :
                                            (gi * TAPS + tap + 1) * 128,
                                        ]
                                        nc.tensor.matmul(
                                            out=psv[
                                                :,
                                                ck * CHUNK_R : (ck + 1) * CHUNK_R,
                                                :,
                                            ],
                                            lhsT=w_ap,
                                            rhs=Gv[
                                                :,
                                                cr + dr : cr + dr + CHUNK_R,
                                                dc : dc + N,
                                            ],
                                            start=(tap == 0),
                                            stop=(tap == TAPS - 1),
                                        )
                                ov = o_tiles[h][:].rearrange(
                                    "p (r c ch) -> p r c ch", c=N, ch=C
                                )
                                # Evacuate PSUM -> interleaved bf16 output.
                                # Alternate ACT/DVE so neither engine's
                                # stride-penalized writes become critical.
                                if ch % 2 == 0:
                                    nc.scalar.activation(
                                        ov[:, :, :, ch], psv, Copy, scale=1.0
                                    )
                                else:
                                    nc.vector.tensor_copy(
                                        out=ov[:, :, :, ch], in_=psv
                                    )

                        for h in range(HALVES if do_store else 0):
                            dst = out[
                                g * GS : (g + 1) * GS, h * HR : (h + 1) * HR, :, :
                            ]
                            # Stores issue from the sync (SP) engine only: a
                            # store on nc.scalar would make the ACT queue
                            # stall on the o-tile semaphore, blocking the
                            # PSUM evacuations behind it.
                            nc.sync.dma_start(
                                out=dst,
                                in_=o_tiles[h][:].rearrange(
                                    "p (r c ch) -> p r c ch", c=N, ch=C
                                ),
                            )

            if loop_iters > 1:
                with tc.For_i(0, loop_iters):
                    _body()
            else:
                _body()
    nc.finalize()
    return nc


def get_nc():
    if "nc" not in _NC_CACHE:
        _NC_CACHE["nc"] = _build_nc()
    return _NC_CACHE["nc"]


def make_core_inputs(padded_obj, positions):
    """Host-side prep: shard + window metadata. Returns list of in_maps."""
    padded_obj = np.asarray(padded_obj, dtype=np.float32)
    positions = np.asarray(positions, dtype=np.float32)
    ox = positions[:, 0, 0, :]  # [B, C] column offsets
    oy = positions[:, 0, 1, :]  # [B, C] row offsets
    c0 = np.float32((M - N) // 2)
    sx = (c0 + ox).astype(np.float32)
    sy = (c0 + oy).astype(np.float32)
    x0 = np.floor(sx).astype(np.int32)
    y0 = np.floor(sy).astype(np.int32)
    fx = (sx - x0.astype(np.float32)).astype(np.float32)
    fy = (sy - y0.astype(np.float32)).astype(np.float32)

    p = np.arange(GS)
    in_maps = []
    for core in range(NCORES):
        s = slice(core * BC, (core + 1) * BC)
        img_c = np.empty((BC * MC * MC + PAD, 1), NP_BF16)
        img_c[: BC * MC * MC, 0] = (
            np.ascontiguousarray(
                padded_obj[s, CROP0 : CROP0 + MC, CROP0 : CROP0 + MC, 0]
            )
            .reshape(-1)
            .astype(NP_BF16)
        )
        img_c[BC * MC * MC :, 0] = 0
        y0c, x0c = y0[s] - CROP0, x0[s] - CROP0
        fyc, fxc = fy[s], fx[s]
        idx_c = np.empty((128, NG), np.int32)
        w_c = np.zeros((128, NG * TAPS * 128), np.float32)
        for g in range(GROUPS):
            b_loc = g * GS + p  # [128] sample index within core
            for ch in range(C):
                gi = g * C + ch
                idx_c[:, gi] = (
                    b_loc * MC + y0c[b_loc, ch]
                ) * MC + x0c[b_loc, ch]
                fyv, fxv = fyc[b_loc, ch], fxc[b_loc, ch]
                taps = [
                    (1.0 - fyv) * (1.0 - fxv),
                    (1.0 - fyv) * fxv,
                    fyv * (1.0 - fxv),
                    fyv * fxv,
                ]
                for tap in range(TAPS):
                    w_c[p, (gi * TAPS + tap) * 128 + p] = taps[tap]
        in_maps.append(
            {"img": img_c, "idx": idx_c, "wdiag": w_c.astype(NP_BF16)}
        )
    return in_maps


def _make_runner(nc):
    """Build a persistent jitted SPMD executor for `nc` (compiles once).

    Mirrors concourse.bass2jax.run_bass_via_pjrt but caches the jitted
    function and keeps the zero output buffers resident on device, so
    repeated kernel() calls ship only the actual inputs.
    """
    import jax
    from jax.sharding import Mesh, PartitionSpec, NamedSharding
    from jax.experimental.shard_map import shard_map
    from concourse import bass2jax, mybir as mb

    bass2jax.install_neuronx_cc_hook()
    assert not nc.dbg_callbacks, "dbg callbacks unsupported under axon"

    extra_in_maps = {}
    if nc.dbg_addr is not None:
        extra_in_maps[nc.dbg_addr.name] = np.zeros((1, 2), np.uint32)
    partition_name = nc.partition_id_tensor.name if nc.partition_id_tensor else None

    in_names, out_names, out_avals = [], [], []
    for alloc in nc.m.functions[0].allocations:
        if not isinstance(alloc, mb.MemoryLocationSet):
            continue
        name = alloc.memorylocations[0].name
        if alloc.kind == "ExternalInput":
            if name != partition_name:
                in_names.append(name)
        elif alloc.kind == "ExternalOutput":
            out_names.append(name)
            out_avals.append(
                jax.core.ShapedArray(tuple(alloc.tensor_shape), mb.dt.np(alloc.dtype))
            )
    n_params = len(in_names)
    n_outs = len(out_avals)
    all_names = in_names + out_names
    if partition_name is not None:
        all_names = all_names + [partition_name]

    def _body(*args):
        operands = list(args)
        if partition_name is not None:
            operands.append(bass2jax.partition_id_tensor())
        outs = bass2jax._bass_exec_p.bind(
            *operands,
            out_avals=tuple(out_avals),
            in_names=tuple(all_names),
            out_names=tuple(out_names),
            lowering_input_output_aliases=(),
            sim_require_finite=True,
            sim_require_nnan=True,
            nc=nc,
        )
        return tuple(outs)

    devices = jax.devices()[:NCORES]
    mesh = Mesh(np.asarray(devices), ("core",))
    in_specs = (PartitionSpec("core"),) * (n_params + n_outs)
    out_specs = (PartitionSpec("core"),) * n_outs
    sharded = jax.jit(
        shard_map(_body, mesh=mesh, in_specs=in_specs, out_specs=out_specs,
                  check_rep=False),
        keep_unused=True,
    )
    sh = NamedSharding(mesh, PartitionSpec("core"))
    zeros_cache = {}

    def _device_zeros():
        # The NEFF writes every output element; the zero operands are only
        # buffer placeholders. Keep them resident on device across calls.
        if "z" not in zeros_cache:
            zeros_cache["z"] = [
                jax.device_put(
                    np.zeros((NCORES * a.shape[0], *a.shape[1:]), a.dtype), sh
                )
                for a in out_avals
            ]
        return zeros_cache["z"]

    def run(in_maps, device_only=False, device_inputs=None):
        if device_inputs is None:
            if extra_in_maps:
                in_maps = [{**m, **extra_in_maps} for m in in_maps]
            device_inputs = [
                np.concatenate([np.asarray(m[name]) for m in in_maps], axis=0)
                for name in in_names
            ] + _device_zeros()
        out_arrs = sharded(*device_inputs)
        if device_only:
            jax.block_until_ready(out_arrs)
            return None
        return {name: np.asarray(out_arrs[i]) for i, name in enumerate(out_names)}

    def put_inputs(in_maps):
        """Place concat inputs + zero outputs on device once (for benching)."""
        if extra_in_maps:
            in_maps = [{**m, **extra_in_maps} for m in in_maps]
        arrs = [
            jax.device_put(
                np.concatenate([np.asarray(m[name]) for m in in_maps], axis=0), sh
            )
            for name in in_names
        ] + _device_zeros()
        jax.block_until_ready(arrs)
        return arrs

    run.put_inputs = put_inputs
    return run


def get_runner():
    if "run" not in _NC_CACHE:
        _NC_CACHE["run"] = _make_runner(get_nc())
    return _NC_CACHE["run"]


def kernel(padded_obj, positions, N=None):
    assert padded_obj.shape == (B, M, M, 1), padded_obj.shape
    in_maps = make_core_inputs(padded_obj, positions)
    out = get_runner()(in_maps)["out"]
    return np.ascontiguousarray(out.astype(np.float32))


# revision 26
# speedup vs baseline: 1.6278x; 1.3390x over previous
"""Bass/Trainium2 kernel for ExtractPatchesPosition (bilinear patch extraction).

Problem: padded_obj (B=2048, 128, 128, 1) f32, positions (B, 1, 2, C=4) ->
patches (B, 64, 64, 4): per (sample, channel), sample a translated 64x64 grid
out(r,c) = img(r + 32 + oy, c + 32 + ox) with bilinear interpolation.
|offset| <= 20 and margin 32, so samples never leave the image: the patch is
the (65 x 65) window at integer origin (y0, x0) = floor(32 + (oy, ox)) blended
with the separable 2-tap weights (fy, fx).

Sharding: pure data parallel, batch over 8 cores (256 samples/core).

Design notes (what measurement established):
  - HBM (~358 GB/s/NC) is the roofline. SWDGE indirect gathers support only
    ONE element offset per partition with a uniform contiguous run (verified
    on HW: multi-entry offset APs silently use entry 0), so per-channel
    window gathers can't crop columns; read traffic is fixed at ~2x the
    image per channel sweep. Both DMA directions are instead halved by
    dtype: the image is cast to bf16 on the HOST and gathered as bf16
    (17.0 MB/core, measured full-rate ~340 GB/s), and the output is stored
    bf16 (8.4 MB/core) then upcast on the host. End-to-end bf16 rounding
    keeps rel err ~7e-3 vs the 2e-2 gate.
  - DVE is NOT viable for the blend at this size: scalar_tensor_tensor has
    no 2x packed mode (measured 1 el/cycle @0.96 GHz) and the
    channel-interleaved output write is ~2x slower still -> 111 us/iter on
    DVE alone. The blend instead runs on the TENSOR engine as 4 accumulating
    diagonal matmuls per (group, channel): a diag(w) stationary matrix is a
    per-partition scalar multiply, PSUM accumulates the 4 bilinear taps, and
    ACT (stride-agnostic, 1 el/cycle @1.2 GHz) evacuates PSUM into the
    channel-interleaved bf16 output tile.

Per (group g, channel ch):
  1. SWDGE indirect gather: per partition 65*128 bf16 from flat offset
     ((g*128+p)*128 + y0)*128 + x0 (one 16.6 KB descriptor per partition,
     both data-dependent shifts absorbed into the element-granularity start).
  2. per half h, per PSUM chunk (16 rows): 4 matmuls
     psum[r,c] += diag(w_tap) @ G[r0+dr+r, dc+c], taps (dr,dc) in {0,1}^2,
     w_tap = host-computed {(1-fy)(1-fx), (1-fy)fx, fy(1-fx), fy fx}.
  3. ACT evacuates psum (32x64) -> o[g][:, r, c, ch] (bf16, interleaved).
  4. after ch=3: two HWDGE stores (one per half) of
     out[g*128:(g+1)*128, 32h:32h+32, :, :]   (2MB bf16, 16KB descriptors)

Window metadata (int origins, per-tap diagonal weights) is computed on host
from `positions` (O(B*C) work) and passed as small input tensors; all
O(B*N*N*C) data movement and math runs on device.
"""

import numpy as np
import ml_dtypes

import concourse.bacc as bacc
import concourse.tile as tile
from concourse import mybir
from concourse.bass import IndirectOffsetOnAxis

B, M, N, C = 2048, 128, 64, 4
NCORES = 8
BC = B // NCORES          # 256 samples per core
GS = 128                  # samples per group (one per partition)
GROUPS = BC // GS         # 2 groups
HALVES = 2                # row halves per group (PSUM/store granularity)
HR = N // HALVES          # 32 output rows per half
WR = N + 1                # 65 gathered window rows per (group, channel)
# Window origins are bounded: y0, x0 = floor(32 + offset) with |offset|<=20,
# so every 65x65 window lies inside rows/cols [12, 117) of the 128x128 image.
# The host crops to that 105x105 region before shipping; gather runs shrink
# from 65*128 to 64*105+65 elements (-18% HBM reads).
CROP0 = 12                # first row/col of the cropped region
MC = 105                  # cropped image width/height
RUNW = WR * MC            # 6825 gathered elements per gather unit
PAD = 64                  # DRAM tail pad: last sample's max-offset window
                          # run reads up to 40 els past the cropped image
NG = GROUPS * C           # 8 gather units
TAPS = 4                  # bilinear taps (dr, dc) in {0,1}^2
MMF = 512                 # matmul free-element limit (s3d3_mm_num_elements)
CHUNK_R = MMF // N        # 8 window rows per matmul chunk
F32 = mybir.dt.float32
BF16 = mybir.dt.bfloat16
NP_BF16 = ml_dtypes.bfloat16
Copy = mybir.ActivationFunctionType.Copy

_NC_CACHE = {}


def _build_nc(reps=1, loop_iters=1, do_gather=True, do_compute=True,
              do_store=True, gbufs=6, obufs=4):
    """Build the Bass module. reps>1 replicates the whole pipeline inside the
    NEFF (same tiles, same output) and loop_iters>1 wraps that body in a
    hardware loop — both used only for steady-state timing (total iterations
    per NEFF execution = reps * loop_iters). The do_* flags disable pipeline
    stages for isolation micro-benchmarks (disabled stages read from
    once-initialized shared tiles so every access stays legal)."""
    nc = bacc.Bacc("TRN2")
    img = nc.declare_dram_parameter(
        "img", [BC * MC * MC + PAD, 1], BF16, isOutput=False
    )
    idx = nc.declare_dram_parameter("idx", [128, NG], mybir.dt.int32, isOutput=False)
    wdiag = nc.declare_dram_parameter(
        "wdiag", [128, NG * TAPS * 128], BF16, isOutput=False
    )
    out = nc.declare_dram_parameter("out", [BC, N, N, C], BF16, isOutput=True)

    with tile.TileContext(nc) as tc:
        with (
            tc.tile_pool(name="singles", bufs=1) as singles,
            tc.tile_pool(name="gpool", bufs=gbufs) as gpool,
            tc.tile_pool(name="opool", bufs=obufs) as opool,
            tc.tile_pool(name="ppool", bufs=2, space="PSUM") as ppool,
        ):
            idx_sb = singles.tile([128, NG], mybir.dt.int32)
            w_sb = singles.tile([128, NG * TAPS * 128], BF16)
            nc.sync.dma_start(idx_sb[:], idx[:])
            nc.sync.dma_start(w_sb[:], wdiag[:])
            if not do_gather:
                G_shared = singles.tile([128, RUNW], BF16)
                nc.sync.dma_start(
                    G_shared[:],
                    img[: 128 * RUNW, :].rearrange("(p x) 1 -> p x", p=128),
                )
            if not do_compute:
                o_shared = [
                    singles.tile([128, HR * N * C], BF16, name=f"osh{h}")
                    for h in range(HALVES)
                ]
                for h in range(HALVES):
                    nc.sync.dma_start(
                        o_shared[h][:],
                        img[: 128 * HR * N * C, :].rearrange(
                            "(p x) 1 -> p x", p=128
                        ),
                    )

            def _body():
                for rep in range(reps):
                    for g in range(GROUPS):
                        if do_compute:
                            o_tiles = [
                                opool.tile(
                                    [128, HR * N * C], BF16, tag="o", name=f"o{h}"
                                )
                                for h in range(HALVES)
                            ]
                        else:
                            o_tiles = o_shared
                        for ch in range(C):
                            gi = g * C + ch
                            if do_gather:
                                G = gpool.tile([128, RUNW], BF16, tag="G")
                                nc.gpsimd.indirect_dma_start(
                                    out=G[:],
                                    out_offset=None,
                                    in_=img[:],
                                    in_offset=IndirectOffsetOnAxis(
                                        ap=idx_sb[:, gi : gi + 1], axis=0
                                    ),
                                )
                            else:
                                G = G_shared
                            Gv = G[:].rearrange("p (r x) -> p r x", x=MC)

                            for h in range(HALVES if do_compute else 0):
                                r0 = h * HR
                                ps = ppool.tile([128, HR * N], F32, tag="ps")
                                psv = ps[:].rearrange("p (r x) -> p r x", x=N)
                                for ck in range(HR // CHUNK_R):
                                    cr = r0 + ck * CHUNK_R
                                    for tap in range(TAPS):
                                        dr, dc = tap >> 1, tap & 1
                                        w_ap = w_sb[
                                            :,
                                            (gi * TAPS + tap) * 128 :
                                            (gi * TAPS + tap + 1) * 128,
                                        ]
                                        nc.tensor.matmul(
                                            out=psv[
                                                :,
                                                ck * CHUNK_R : (ck + 1) * CHUNK_R,
                                                :,
                                            ],
                                            lhsT=w_ap,
                                            rhs=Gv[
                                                :,
                                                cr + dr : cr + dr + CHUNK_R,
                                                dc : dc + N,
                                            ],
                                            start=(tap == 0),
                                            stop=(tap == TAPS - 1),
                                        )
                                ov = o_tiles[h][:].rearrange(
                                    "p (r c ch) -> p r c ch", c=N, ch=C
                                )
                                # Evacuate PSUM -> interleaved bf16 output.
                                # Alternate ACT/DVE so neither engine's
                                # stride-penalized writes become critical.
                                if ch % 2 == 0:
                                    nc.scalar.activation(
                                        ov[:, :, :, ch], psv, Copy, scale=1.0
                                    )
                                else:
                                    nc.vector.tensor_copy(
                                        out=ov[:, :, :, ch], in_=psv
                                    )

                        for h in range(HALVES if do_store else 0):
                            dst = out[
                                g * GS : (g + 1) * GS, h * HR : (h + 1) * HR, :, :
                            ]
                            # Stores issue from the sync (SP) engine only: a
                            # store on nc.scalar would make the ACT queue
                            # stall on the o-tile semaphore, blocking the
                            # PSUM evacuations behind it.
                            nc.sync.dma_start(
                                out=dst,
                                in_=o_tiles[h][:].rearrange(
                                    "p (r c ch) -> p r c ch", c=N, ch=C
                                ),
                            )

            if loop_iters > 1:
                with tc.For_i(0, loop_iters):
                    _body()
            else:
                _body()
    nc.finalize()
    return nc


def get_nc():
    if "nc" not in _NC_CACHE:
        _NC_CACHE["nc"] = _build_nc()
    return _NC_CACHE["nc"]


def make_core_inputs(padded_obj, positions):
    """Host-side prep: shard + window metadata. Returns list of in_maps."""
    padded_obj = np.asarray(padded_obj, dtype=np.float32)
    positions = np.asarray(positions, dtype=np.float32)
    ox = positions[:, 0, 0, :]  # [B, C] column offsets
    oy = positions[:, 0, 1, :]  # [B, C] row offsets
    c0 = np.float32((M - N) // 2)
    sx = (c0 + ox).astype(np.float32)
    sy = (c0 + oy).astype(np.float32)
    x0 = np.floor(sx).astype(np.int32)
    y0 = np.floor(sy).astype(np.int32)
    fx = (sx - x0.astype(np.float32)).astype(np.float32)
    fy = (sy - y0.astype(np.float32)).astype(np.float32)

    p = np.arange(GS)
    in_maps = []
    for core in range(NCORES):
        s = slice(core * BC, (core + 1) * BC)
        img_c = np.empty((BC * MC * MC + PAD, 1), NP_BF16)
        img_c[: BC * MC * MC, 0] = (
            np.ascontiguousarray(
                padded_obj[s, CROP0 : CROP0 + MC, CROP0 : CROP0 + MC, 0]
            )
            .reshape(-1)
            .astype(NP_BF16)
        )
        img_c[BC * MC * MC :, 0] = 0
        y0c, x0c = y0[s] - CROP0, x0[s] - CROP0
        fyc, fxc = fy[s], fx[s]
        idx_c = np.empty((128, NG), np.int32)
        w_c = np.zeros((128, NG * TAPS * 128), np.float32)
        for g in range(GROUPS):
            b_loc = g * GS + p  # [128] sample index within core
            for ch in range(C):
                gi = g * C + ch
                idx_c[:, gi] = (
                    b_loc * MC + y0c[b_loc, ch]
                ) * MC + x0c[b_loc, ch]
                fyv, fxv = fyc[b_loc, ch], fxc[b_loc, ch]
                taps = [
                    (1.0 - fyv) * (1.0 - fxv),
                    (1.0 - fyv) * fxv,
                    fyv * (1.0 - fxv),
                    fyv * fxv,
                ]
                for tap in range(TAPS):
                    w_c[p, (gi * TAPS + tap) * 128 + p] = taps[tap]
        in_maps.append(
            {"img": img_c, "idx": idx_c, "wdiag": w_c.astype(NP_BF16)}
        )
    return in_maps


def _make_runner(nc):
    """Build a persistent jitted SPMD executor for `nc` (compiles once).

    Mirrors concourse.bass2jax.run_bass_via_pjrt but caches the jitted
    function and keeps the zero output buffers resident on device, so
    repeated kernel() calls ship only the actual inputs.
    """
    import jax
    from jax.sharding import Mesh, PartitionSpec, NamedSharding
    from jax.experimental.shard_map import shard_map
    from concourse import bass2jax, mybir as mb

    bass2jax.install_neuronx_cc_hook()
    assert not nc.dbg_callbacks, "dbg callbacks unsupported under axon"

    extra_in_maps = {}
    if nc.dbg_addr is not None:
        extra_in_maps[nc.dbg_addr.name] = np.zeros((1, 2), np.uint32)
    partition_name = nc.partition_id_tensor.name if nc.partition_id_tensor else None

    in_names, out_names, out_avals = [], [], []
    for alloc in nc.m.functions[0].allocations:
        if not isinstance(alloc, mb.MemoryLocationSet):
            continue
        name = alloc.memorylocations[0].name
        if alloc.kind == "ExternalInput":
            if name != partition_name:
                in_names.append(name)
        elif alloc.kind == "ExternalOutput":
            out_names.append(name)
            out_avals.append(
                jax.core.ShapedArray(tuple(alloc.tensor_shape), mb.dt.np(alloc.dtype))
            )
    n_params = len(in_names)
    n_outs = len(out_avals)
    all_names = in_names + out_names
    if partition_name is not None:
        all_names = all_names + [partition_name]

    def _body(*args):
        operands = list(args)
        if partition_name is not None:
            operands.append(bass2jax.partition_id_tensor())
        outs = bass2jax._bass_exec_p.bind(
            *operands,
            out_avals=tuple(out_avals),
            in_names=tuple(all_names),
            out_names=tuple(out_names),
            lowering_input_output_aliases=(),
            sim_require_finite=True,
            sim_require_nnan=True,
            nc=nc,
        )
        return tuple(outs)

    devices = jax.devices()[:NCORES]
    mesh = Mesh(np.asarray(devices), ("core",))
    in_specs = (PartitionSpec("core"),) * (n_params + n_outs)
    out_specs = (PartitionSpec("core"),) * n_outs
    sharded = jax.jit(
        shard_map(_body, mesh=mesh, in_specs=in_specs, out_specs=out_specs,
                  check_rep=False),
        keep_unused=True,
    )
    sh = NamedSharding(mesh, PartitionSpec("core"))
    zeros_cache = {}

    def _device_zeros():
        # The NEFF writes every output element; the zero operands are only
        # buffer placeholders. Keep them resident on device across calls.
        if "z" not in zeros_cache:
            zeros_cache["z"] = [
                jax.device_put(
                    np.zeros((NCORES * a.shape[0], *a.shape[1:]), a.dtype), sh
                )
                for a in out_avals
            ]
        return zeros_cache["z"]

    def run(in_maps, device_only=False, device_inputs=None):
        if device_inputs is None:
            if extra_in_maps:
                in_maps = [{**m, **extra_in_maps} for m in in_maps]
            device_inputs = [
                np.concatenate([np.asarray(m[name]) for m in in_maps], axis=0)
                for name in in_names
            ] + _device_zeros()
        out_arrs = sharded(*device_inputs)
        if device_only:
            jax.block_until_ready(out_arrs)
            return None
        return {name: np.asarray(out_arrs[i]) for i, name in enumerate(out_names)}

    def put_inputs(in_maps):
        """Place concat inputs + zero outputs on device once (for benching)."""
        if extra_in_maps:
            in_maps = [{**m, **extra_in_maps} for m in in_maps]
        arrs = [
            jax.device_put(
                np.concatenate([np.asarray(m[name]) for m in in_maps], axis=0), sh
            )
            for name in in_names
        ] + _device_zeros()
        jax.block_until_ready(arrs)
        return arrs

    run.put_inputs = put_inputs
    return run


def get_runner():
    if "run" not in _NC_CACHE:
        _NC_CACHE["run"] = _make_runner(get_nc())
    return _NC_CACHE["run"]


def kernel(padded_obj, positions, N=None):
    assert padded_obj.shape == (B, M, M, 1), padded_obj.shape
    in_maps = make_core_inputs(padded_obj, positions)
    out = get_runner()(in_maps)["out"]
    return np.ascontiguousarray(out.astype(np.float32))


# revision 30
# speedup vs baseline: 1.6383x; 1.0065x over previous
"""Bass/Trainium2 kernel for ExtractPatchesPosition (bilinear patch extraction).

Problem: padded_obj (B=2048, 128, 128, 1) f32, positions (B, 1, 2, C=4) ->
patches (B, 64, 64, 4): per (sample, channel), sample a translated 64x64 grid
out(r,c) = img(r + 32 + oy, c + 32 + ox) with bilinear interpolation.
|offset| <= 20 and margin 32, so samples never leave the image: the patch is
the (65 x 65) window at integer origin (y0, x0) = floor(32 + (oy, ox)) blended
with the separable 2-tap weights (fy, fx).

Sharding: pure data parallel, batch over 8 cores (256 samples/core).

Design notes (what measurement established):
  - HBM (~358 GB/s/NC) is the roofline. SWDGE indirect gathers support only
    ONE element offset per partition with a uniform contiguous run (verified
    on HW: multi-entry offset APs silently use entry 0), so per-channel
    window gathers can't crop columns; read traffic is fixed at ~2x the
    image per channel sweep. Both DMA directions are instead halved by
    dtype: the image is cast to bf16 on the HOST and gathered as bf16
    (17.0 MB/core, measured full-rate ~340 GB/s), and the output is stored
    bf16 (8.4 MB/core) then upcast on the host. End-to-end bf16 rounding
    keeps rel err ~7e-3 vs the 2e-2 gate.
  - DVE is NOT viable for the blend at this size: scalar_tensor_tensor has
    no 2x packed mode (measured 1 el/cycle @0.96 GHz) and the
    channel-interleaved output write is ~2x slower still -> 111 us/iter on
    DVE alone. The blend instead runs on the TENSOR engine as 4 accumulating
    diagonal matmuls per (group, channel): a diag(w) stationary matrix is a
    per-partition scalar multiply, PSUM accumulates the 4 bilinear taps, and
    ACT (stride-agnostic, 1 el/cycle @1.2 GHz) evacuates PSUM into the
    channel-interleaved bf16 output tile.

Per (group g, channel ch):
  1. SWDGE indirect gather: per partition 65*128 bf16 from flat offset
     ((g*128+p)*128 + y0)*128 + x0 (one 16.6 KB descriptor per partition,
     both data-dependent shifts absorbed into the element-granularity start).
  2. per half h, per PSUM chunk (16 rows): 4 matmuls
     psum[r,c] += diag(w_tap) @ G[r0+dr+r, dc+c], taps (dr,dc) in {0,1}^2,
     w_tap = host-computed {(1-fy)(1-fx), (1-fy)fx, fy(1-fx), fy fx}.
  3. ACT evacuates psum (32x64) -> o[g][:, r, c, ch] (bf16, interleaved).
  4. after ch=3: two HWDGE stores (one per half) of
     out[g*128:(g+1)*128, 32h:32h+32, :, :]   (2MB bf16, 16KB descriptors)

Window metadata (int origins, per-tap diagonal weights) is computed on host
from `positions` (O(B*C) work) and passed as small input tensors; all
O(B*N*N*C) data movement and math runs on device.
"""

import numpy as np
import ml_dtypes

import concourse.bacc as bacc
import concourse.tile as tile
from concourse import mybir
from concourse.bass import IndirectOffsetOnAxis

B, M, N, C = 2048, 128, 64, 4
NCORES = 8
BC = B // NCORES          # 256 samples per core
GS = 128                  # samples per group (one per partition)
GROUPS = BC // GS         # 2 groups
HALVES = 2                # row halves per group (PSUM/store granularity)
HR = N // HALVES          # 32 output rows per half
WR = N + 1                # 65 gathered window rows per (group, channel)
# Window origins are bounded: y0, x0 = floor(32 + offset) with |offset|<=20,
# so every 65x65 window lies inside rows/cols [12, 117) of the 128x128 image.
# The host crops to that 105x105 region AND ships NCOPY re-strided copies at
# staggered column offsets (0, 8, .., 32) of width 73; each gather reads from
# the copy whose 73 columns contain its window, so the contiguous per-
# partition run shrinks from 65*128=8320 to 65*73=4745 elements (gather HBM
# reads drop 17.0 MB -> 9.7 MB per core).
CROP0 = 12                # first row/col of the cropped region
MC = 105                  # cropped image height (rows)
WC = 73                   # width of each re-strided copy
DSTEP = 8                 # column stagger between copies (= WC - 65)
NCOPY = 5                 # staggered copies; covers x0-CROP0 in [0, 40]
COPYSZ = BC * MC * WC     # elements per copy
RUNW = WR * WC            # 4745 gathered elements per gather unit
PAD = 64                  # DRAM tail pad: last sample's max-offset window
                          # run reads a few els past the last copy
NG = GROUPS * C           # 8 gather units
TAPS = 4                  # bilinear taps (dr, dc) in {0,1}^2
MMF = 512                 # matmul free-element limit (s3d3_mm_num_elements)
CHUNK_R = MMF // N        # 8 window rows per matmul chunk
F32 = mybir.dt.float32
BF16 = mybir.dt.bfloat16
NP_BF16 = ml_dtypes.bfloat16
Copy = mybir.ActivationFunctionType.Copy

_NC_CACHE = {}


def _build_nc(reps=1, loop_iters=1, do_gather=True, do_compute=True,
              do_store=True, gbufs=6, obufs=4):
    """Build the Bass module. reps>1 replicates the whole pipeline inside the
    NEFF (same tiles, same output) and loop_iters>1 wraps that body in a
    hardware loop — both used only for steady-state timing (total iterations
    per NEFF execution = reps * loop_iters). The do_* flags disable pipeline
    stages for isolation micro-benchmarks (disabled stages read from
    once-initialized shared tiles so every access stays legal)."""
    nc = bacc.Bacc("TRN2")
    img = nc.declare_dram_parameter(
        "img", [NCOPY * COPYSZ + PAD, 1], BF16, isOutput=False
    )
    idx = nc.declare_dram_parameter("idx", [128, NG], mybir.dt.int32, isOutput=False)
    wdiag = nc.declare_dram_parameter(
        "wdiag", [128, NG * TAPS * 128], BF16, isOutput=False
    )
    out = nc.declare_dram_parameter("out", [BC, N, N, C], BF16, isOutput=True)

    with tile.TileContext(nc) as tc:
        with (
            tc.tile_pool(name="singles", bufs=1) as singles,
            tc.tile_pool(name="gpool", bufs=gbufs) as gpool,
            tc.tile_pool(name="opool", bufs=obufs) as opool,
            tc.tile_pool(name="ppool", bufs=2, space="PSUM") as ppool,
        ):
            idx_sb = singles.tile([128, NG], mybir.dt.int32)
            w_sb = singles.tile([128, NG * TAPS * 128], BF16)
            nc.sync.dma_start(idx_sb[:], idx[:])
            nc.sync.dma_start(w_sb[:], wdiag[:])
            if not do_gather:
                G_shared = singles.tile([128, RUNW], BF16)
                nc.sync.dma_start(
                    G_shared[:],
                    img[: 128 * RUNW, :].rearrange("(p x) 1 -> p x", p=128),
                )
            if not do_compute:
                o_shared = [
                    singles.tile([128, HR * N * C], BF16, name=f"osh{h}")
                    for h in range(HALVES)
                ]
                for h in range(HALVES):
                    nc.sync.dma_start(
                        o_shared[h][:],
                        img[: 128 * HR * N * C, :].rearrange(
                            "(p x) 1 -> p x", p=128
                        ),
                    )

            def _body():
                for rep in range(reps):
                    for g in range(GROUPS):
                        if do_compute:
                            o_tiles = [
                                opool.tile(
                                    [128, HR * N * C], BF16, tag="o", name=f"o{h}"
                                )
                                for h in range(HALVES)
                            ]
                        else:
                            o_tiles = o_shared
                        for ch in range(C):
                            gi = g * C + ch
                            if do_gather:
                                G = gpool.tile([128, RUNW], BF16, tag="G")
                                nc.gpsimd.indirect_dma_start(
                                    out=G[:],
                                    out_offset=None,
                                    in_=img[:],
                                    in_offset=IndirectOffsetOnAxis(
                                        ap=idx_sb[:, gi : gi + 1], axis=0
                                    ),
                                )
                            else:
                                G = G_shared
                            Gv = G[:].rearrange("p (r x) -> p r x", x=WC)

                            for h in range(HALVES if do_compute else 0):
                                r0 = h * HR
                                ps = ppool.tile([128, HR * N], F32, tag="ps")
                                psv = ps[:].rearrange("p (r x) -> p r x", x=N)
                                for ck in range(HR // CHUNK_R):
                                    cr = r0 + ck * CHUNK_R
                                    for tap in range(TAPS):
                                        dr, dc = tap >> 1, tap & 1
                                        w_ap = w_sb[
                                            :,
                                            (gi * TAPS + tap) * 128 :
                                            (gi * TAPS + tap + 1) * 128,
                                        ]
                                        nc.tensor.matmul(
                                            out=psv[
                                                :,
                                                ck * CHUNK_R : (ck + 1) * CHUNK_R,
                                                :,
                                            ],
                                            lhsT=w_ap,
                                            rhs=Gv[
                                                :,
                                                cr + dr : cr + dr + CHUNK_R,
                                                dc : dc + N,
                                            ],
                                            start=(tap == 0),
                                            stop=(tap == TAPS - 1),
                                        )
                                ov = o_tiles[h][:].rearrange(
                                    "p (r c ch) -> p r c ch", c=N, ch=C
                                )
                                # Evacuate PSUM -> interleaved bf16 output.
                                # Alternate ACT/DVE so neither engine's
                                # stride-penalized writes become critical.
                                if ch % 2 == 0:
                                    nc.scalar.activation(
                                        ov[:, :, :, ch], psv, Copy, scale=1.0
                                    )
                                else:
                                    nc.vector.tensor_copy(
                                        out=ov[:, :, :, ch], in_=psv
                                    )

                        for h in range(HALVES if do_store else 0):
                            dst = out[
                                g * GS : (g + 1) * GS, h * HR : (h + 1) * HR, :, :
                            ]
                            # Stores issue from the sync (SP) engine only: a
                            # store on nc.scalar would make the ACT queue
                            # stall on the o-tile semaphore, blocking the
                            # PSUM evacuations behind it.
                            nc.sync.dma_start(
                                out=dst,
                                in_=o_tiles[h][:].rearrange(
                                    "p (r c ch) -> p r c ch", c=N, ch=C
                                ),
                            )

            if loop_iters > 1:
                with tc.For_i(0, loop_iters):
                    _body()
            else:
                _body()
    nc.finalize()
    return nc


def get_nc():
    if "nc" not in _NC_CACHE:
        _NC_CACHE["nc"] = _build_nc()
    return _NC_CACHE["nc"]


def make_core_inputs(padded_obj, positions):
    """Host-side prep: shard + window metadata. Returns list of in_maps."""
    padded_obj = np.asarray(padded_obj, dtype=np.float32)
    positions = np.asarray(positions, dtype=np.float32)
    ox = positions[:, 0, 0, :]  # [B, C] column offsets
    oy = positions[:, 0, 1, :]  # [B, C] row offsets
    c0 = np.float32((M - N) // 2)
    sx = (c0 + ox).astype(np.float32)
    sy = (c0 + oy).astype(np.float32)
    x0 = np.floor(sx).astype(np.int32)
    y0 = np.floor(sy).astype(np.int32)
    fx = (sx - x0.astype(np.float32)).astype(np.float32)
    fy = (sy - y0.astype(np.float32)).astype(np.float32)

    p = np.arange(GS)
    in_maps = []
    for core in range(NCORES):
        s = slice(core * BC, (core + 1) * BC)
        crop = (
            np.ascontiguousarray(
                padded_obj[s, CROP0 : CROP0 + MC, CROP0 : CROP0 + MC, 0]
            ).astype(NP_BF16)
        )  # [BC, MC, MC]
        img_c = np.empty((NCOPY * COPYSZ + PAD, 1), NP_BF16)
        for k in range(NCOPY):
            img_c[k * COPYSZ : (k + 1) * COPYSZ, 0] = np.ascontiguousarray(
                crop[:, :, k * DSTEP : k * DSTEP + WC]
            ).reshape(-1)
        img_c[NCOPY * COPYSZ :, 0] = 0
        y0c, x0c = y0[s] - CROP0, x0[s] - CROP0
        fyc, fxc = fy[s], fx[s]
        idx_c = np.empty((128, NG), np.int32)
        w_c = np.zeros((128, NG * TAPS * 128), np.float32)
        for g in range(GROUPS):
            b_loc = g * GS + p  # [128] sample index within core
            for ch in range(C):
                gi = g * C + ch
                xq = x0c[b_loc, ch]
                k = np.minimum(NCOPY - 1, xq // DSTEP)
                idx_c[:, gi] = (
                    k * COPYSZ
                    + (b_loc * MC + y0c[b_loc, ch]) * WC
                    + (xq - k * DSTEP)
                )
                fyv, fxv = fyc[b_loc, ch], fxc[b_loc, ch]
                taps = [
                    (1.0 - fyv) * (1.0 - fxv),
                    (1.0 - fyv) * fxv,
                    fyv * (1.0 - fxv),
                    fyv * fxv,
                ]
                for tap in range(TAPS):
                    w_c[p, (gi * TAPS + tap) * 128 + p] = taps[tap]
        in_maps.append(
            {"img": img_c, "idx": idx_c, "wdiag": w_c.astype(NP_BF16)}
        )
    return in_maps


def _make_runner(nc):
    """Build a persistent jitted SPMD executor for `nc` (compiles once).

    Mirrors concourse.bass2jax.run_bass_via_pjrt but caches the jitted
    function and keeps the zero output buffers resident on device, so
    repeated kernel() calls ship only the actual inputs.
    """
    import jax
    from jax.sharding import Mesh, PartitionSpec, NamedSharding
    from jax.experimental.shard_map import shard_map
    from concourse import bass2jax, mybir as mb

    bass2jax.install_neuronx_cc_hook()
    assert not nc.dbg_callbacks, "dbg callbacks unsupported under axon"

    extra_in_maps = {}
    if nc.dbg_addr is not None:
        extra_in_maps[nc.dbg_addr.name] = np.zeros((1, 2), np.uint32)
    partition_name = nc.partition_id_tensor.name if nc.partition_id_tensor else None

    in_names, out_names, out_avals = [], [], []
    for alloc in nc.m.functions[0].allocations:
        if not isinstance(alloc, mb.MemoryLocationSet):
            continue
        name = alloc.memorylocations[0].name
        if alloc.kind == "ExternalInput":
            if name != partition_name:
                in_names.append(name)
        elif alloc.kind == "ExternalOutput":
            out_names.append(name)
            out_avals.append(
                jax.core.ShapedArray(tuple(alloc.tensor_shape), mb.dt.np(alloc.dtype))
            )
    n_params = len(in_names)
    n_outs = len(out_avals)
    all_names = in_names + out_names
    if partition_name is not None:
        all_names = all_names + [partition_name]

    def _body(*args):
        operands = list(args)
        if partition_name is not None:
            operands.append(bass2jax.partition_id_tensor())
        outs = bass2jax._bass_exec_p.bind(
            *operands,
            out_avals=tuple(out_avals),
            in_names=tuple(all_names),
            out_names=tuple(out_names),
            lowering_input_output_aliases=(),
            sim_require_finite=True,
            sim_require_nnan=True,
            nc=nc,
        )
        return tuple(outs)

    devices = jax.devices()[:NCORES]
    mesh = Mesh(np.asarray(devices), ("core",))
    in_specs = (PartitionSpec("core"),) * (n_params + n_outs)
    out_specs = (PartitionSpec("core"),) * n_outs
    sharded = jax.jit(
        shard_map(_body, mesh=mesh, in_specs=in_specs, out_specs=out_specs,
                  check_rep=False),
        keep_unused=True,
    )
    sh = NamedSharding(mesh, PartitionSpec("core"))
    zeros_cache = {}

    def _device_zeros():
        # The NEFF writes every output element; the zero operands are only
        # buffer placeholders. Keep them resident on device across calls.
        if "z" not in zeros_cache:
            zeros_cache["z"] = [
                jax.device_put(
                    np.zeros((NCORES * a.shape[0], *a.shape[1:]), a.dtype), sh
                )
                for a in out_avals
            ]
        return zeros_cache["z"]

    def run(in_maps, device_only=False, device_inputs=None):
        if device_inputs is None:
            if extra_in_maps:
                in_maps = [{**m, **extra_in_maps} for m in in_maps]
            device_inputs = [
                np.concatenate([np.asarray(m[name]) for m in in_maps], axis=0)
                for name in in_names
            ] + _device_zeros()
        out_arrs = sharded(*device_inputs)
        if device_only:
            jax.block_until_ready(out_arrs)
            return None
        return {name: np.asarray(out_arrs[i]) for i, name in enumerate(out_names)}

    def put_inputs(in_maps):
        """Place concat inputs + zero outputs on device once (for benching)."""
        if extra_in_maps:
            in_maps = [{**m, **extra_in_maps} for m in in_maps]
        arrs = [
            jax.device_put(
                np.concatenate([np.asarray(m[name]) for m in in_maps], axis=0), sh
            )
            for name in in_names
        ] + _device_zeros()
        jax.block_until_ready(arrs)
        return arrs

    run.put_inputs = put_inputs
    return run


def get_runner():
    if "run" not in _NC_CACHE:
        _NC_CACHE["run"] = _make_runner(get_nc())
    return _NC_CACHE["run"]


def kernel(padded_obj, positions, N=None):
    assert padded_obj.shape == (B, M, M, 1), padded_obj.shape
    in_maps = make_core_inputs(padded_obj, positions)
    out = get_runner()(in_maps)["out"]
    return np.ascontiguousarray(out.astype(np.float32))
